# revision 29
# baseline (speedup 1.0000x reference)
"""Trainium2 Bass kernel for nn_CogMemBank (episodic memory bank retrieval).

Contract: kernel(**inputs) takes FULL unsharded inputs (as produced by
setup_inputs) and returns the FULL (B, 1, D) output.

Strategy
--------
The reference is a sequential scan over B=128 batch elements, but the bank
update appends the *raw input token* (not a computed value), so the bank
evolution is fully determined by the inputs. The per-element effective memory
(L, D) view is resolved on the host with pure indexing (routing/gather), after
which all 128 retrieval+gate computations are independent.

Device: data-parallel over batch elements, 16 per core on 8 NeuronCores, with
the (replicated) attention/gate/PE-MLP weights streamed from HBM. Activations
live feature-major (features on SBUF partitions, elements on the free dim) so
every linear layer is out^T[oc] = sum_kc W[kc,oc]^T @ x^T[kc] with the weight
chunk as the stationary matmul operand. LayerNorm/softmax cross-partition
reductions use ones-vector matmuls; per-element broadcasts use K=1 matmuls
into PSUM.
"""
import sys
import numpy as np

for _p in ("/opt/trn_rl_repo",):
    if _p not in sys.path:
        sys.path.append(_p)

import ml_dtypes  # noqa: E402
import concourse.bass as bass  # noqa: E402
import concourse.tile as tile  # noqa: E402
from concourse import bacc, mybir  # noqa: E402
from concourse.bass_utils import run_bass_kernel_spmd  # noqa: E402

B, D, L, E = 128, 1024, 16, 4096
FREQ = D // 4          # 256
HALF = FREQ // 2       # 128
NLAYERS = 2
NCORES = 8
EPC = B // NCORES      # 16 elements per core
R = EPC * L            # 256 memory rows per core
DC = D // 128          # 8 feature chunks
HC = (4 * D) // 128    # 32 ffn hidden chunks
F32 = mybir.dt.float32
BF16 = mybir.dt.bfloat16
NP_BF16 = ml_dtypes.bfloat16

_CACHE = {}


# ---------------------------------------------------------------- host side

def _resolve_memories(tokens, episode_ids, timesteps, bank_feats, bank_timesteps, bank_counts):
    """Per-element effective (L, D) memory, (L,) timesteps and count, honoring
    the sequential append/FIFO semantics of the reference scan."""
    feats_state, ts_state, cnt_state = {}, {}, {}
    mems = np.zeros((B, L, D), np.float32)
    mts = np.zeros((B, L), np.float32)
    cnts = np.zeros(B, np.int64)
    for b in range(B):
        eid = int(episode_ids[b])
        if eid not in feats_state:
            feats_state[eid] = np.array(bank_feats[eid], np.float32)
            ts_state[eid] = np.array(bank_timesteps[eid], np.float32)
            cnt_state[eid] = int(bank_counts[eid])
        f, t, c = feats_state[eid], ts_state[eid], cnt_state[eid]
        mems[b] = f
        mts[b] = t
        cnts[b] = c
        tok = tokens[b, 0]
        if c < L:
            f[c] = tok
            t[c] = timesteps[b]
            cnt_state[eid] = c + 1
        else:
            f[:-1] = f[1:]
            f[-1] = tok
            t[:-1] = t[1:]
            t[-1] = timesteps[b]
    return mems, mts, cnts


def _pack_w(Wm, KC, OC, ops):
    """Weight (KC*128, OC*128) -> list of slab arrays (128, ops*KC*128) where
    slab s covers oc in [s*ops, (s+1)*ops) and col = oi*(KC*128) + kc*128 + m,
    i.e. slab[p, oi*KC*128 + kc*128 + m] = W[kc*128+p, (s*ops+oi)*128+m]."""
    Wm = np.asarray(Wm, np.float32)
    arr = Wm.reshape(KC, 128, OC, 128).transpose(2, 1, 0, 3)       # (OC,128,KC,128)
    arr = arr.reshape(OC, 128, KC * 128)
    S = OC // ops
    arr = arr.reshape(S, ops, 128, KC * 128).transpose(0, 2, 1, 3)  # (S,128,ops,KC*128)
    return [np.ascontiguousarray(arr[s].reshape(128, ops * KC * 128).astype(NP_BF16))
            for s in range(S)]


def _pack_b(b, OC):
    return np.ascontiguousarray(np.asarray(b, np.float32).reshape(OC, 128).T)


def _pack_ln(s):
    # (D,) -> (128, 128): [p, c*16+e] = s[c*128+p]  (replicated over e)
    a = np.asarray(s, np.float32).reshape(DC, 128).T          # (128, 8)
    return np.ascontiguousarray(
        np.broadcast_to(a[:, :, None], (128, DC, EPC)).reshape(128, DC * EPC))


def _feature_major(x):
    """(rows, D) -> (128, DC*rows): [p, c*rows + r] = x[r, c*128+p]."""
    rows = x.shape[0]
    return np.ascontiguousarray(
        np.asarray(x, np.float32).reshape(rows, DC, 128).transpose(2, 1, 0).reshape(128, DC * rows))


# ------------------------------------------------------------- bass program

def _build_program(debug=False):
    nc = bacc.Bacc("TRN2", target_bir_lowering=False, debug=False)

    # Register extra const scalars used as activation biases (pi/2 for cos via
    # sin, 1e-6 for the layernorm eps) — same mechanism as the builtin 0.0/1.0.
    def _reg_const(val):
        t = nc.alloc_sbuf_tensor(f"const-f32-{val}", [128, 1], F32)
        nc.gpsimd.memset(t.ap(), val)
        nc.const_aps.aps[(F32, float(val))] = t.ap()

    _reg_const(float(np.pi / 2))
    _reg_const(1e-6)
    nc.all_engine_barrier()

    dr = {}
    dshape = {}

    def din(name, shape, dt=F32):
        dr[name] = nc.dram_tensor(name, list(shape), dt, kind="ExternalInput")
        dshape[name] = tuple(shape)

    def dout(name, shape):
        dr[name] = nc.dram_tensor(name, list(shape), F32, kind="ExternalOutput")
        dshape[name] = tuple(shape)

    # per-core data
    din("mems", (128, DC * R), BF16)
    din("toks", (128, DC * EPC))
    din("toks_bf", (128, DC * EPC), BF16)
    din("tsrow", (1, R))
    din("amask", (1, R))
    din("flag", (128, EPC))
    # shared constants
    din("freqs", (1, HALF))
    din("onesrow", (1, 128))
    din("onescol", (128, 1))
    # weights (slabbed) + biases + ln params
    WSPECS = {"m1w": (2, 8, 8), "m2w": (8, 8, 4), "gw": (16, 8, 2)}
    for i in range(NLAYERS):
        WSPECS[f"qw{i}"] = (8, 8, 4)
        WSPECS[f"kw{i}"] = (8, 8, 4)
        WSPECS[f"vw{i}"] = (8, 8, 4)
        WSPECS[f"f1w{i}"] = (8, 32, 4)
        WSPECS[f"f2w{i}"] = (32, 8, 1)
    for name, (KC, OC, ops) in WSPECS.items():
        for s in range(OC // ops):
            din(f"{name}_{s}", (128, ops * KC * 128), BF16)
    din("m1b", (128, 8))
    din("m2b", (128, 8))
    din("gb", (128, 8))
    for i in range(NLAYERS):
        for nm in ("qb", "kb", "vb", "f2b"):
            din(f"{nm}{i}", (128, 8))
        din(f"f1b{i}", (128, 32))
        for nm in ("ln1s", "ln1b", "ln2s", "ln2b"):
            din(f"{nm}{i}", (128, DC * EPC))

    dout("outp", (128, DC * EPC))
    if debug:
        for nm, shape in [("dbg_k0", (128, DC * R)),
                          ("dbg_v0", (128, DC * R)), ("dbg_wn0", (1, R)),
                          ("dbg_x0a", (128, DC * EPC)), ("dbg_x0b", (128, DC * EPC)),
                          ("dbg_retr", (128, DC * EPC)),
                          ("dbg_q0", (128, DC * EPC)), ("dbg_logit0", (1, R))]:
            dout(nm, shape)
        for nm, shape in [("dbg_pe", (128, DC * R)), ("dbg_hml", (128, DC * R))]:
            dr[nm] = nc.dram_tensor(nm, list(shape), BF16, kind="ExternalOutput")
            dshape[nm] = tuple(shape)

    AF = mybir.ActivationFunctionType
    OP = mybir.AluOpType

    with tile.TileContext(nc) as tc:
        from contextlib import ExitStack
        with ExitStack() as ctx:
            const = ctx.enter_context(tc.tile_pool(name="const", bufs=1))
            act = ctx.enter_context(tc.tile_pool(name="act", bufs=1))
            act2 = ctx.enter_context(tc.tile_pool(name="act2", bufs=2))
            wpool = ctx.enter_context(tc.tile_pool(name="w", bufs=12))
            pp256 = ctx.enter_context(tc.tile_pool(name="pp256", bufs=3, space="PSUM"))
            pprow = ctx.enter_context(tc.tile_pool(name="pprow", bufs=2, space="PSUM"))
            pp16 = ctx.enter_context(tc.tile_pool(name="pp16", bufs=2, space="PSUM"))
            ppmr = ctx.enter_context(tc.tile_pool(name="ppmr", bufs=1, space="PSUM"))

            dma_engines = [nc.sync, nc.scalar, nc.gpsimd]
            dma_rr = [0]

            def next_eng():
                e = dma_engines[dma_rr[0] % len(dma_engines)]
                dma_rr[0] += 1
                return e

            def cload(name, shape, dt=F32):
                t = const.tile(list(shape), dt, tag=name, name=f"c_{name}")
                next_eng().dma_start(t[:, :], dr[name].ap())
                return t

            mems = cload("mems", (128, DC * R), BF16)
            toks = cload("toks", (128, DC * EPC))
            toks_bf = cload("toks_bf", (128, DC * EPC), BF16)
            tsrow = cload("tsrow", (1, R))
            amask = cload("amask", (1, R))
            flag = cload("flag", (128, EPC))
            freqs = cload("freqs", (1, HALF))
            onesrow = cload("onesrow", (1, 128))
            onescol = cload("onescol", (128, 1))
            m1b = cload("m1b", (128, 8))
            m2b = cload("m2b", (128, 8))
            gb = cload("gb", (128, 8))
            lb = {}
            for i in range(NLAYERS):
                for nm in ("qb", "kb", "vb", "f2b"):
                    lb[f"{nm}{i}"] = cload(f"{nm}{i}", (128, 8))
                lb[f"f1b{i}"] = cload(f"f1b{i}", (128, 32))
                for nm in ("ln1s", "ln1b", "ln2s", "ln2b"):
                    lb[f"{nm}{i}"] = cload(f"{nm}{i}", (128, DC * EPC))

            def wslab(name):
                cols = dshape[name][1]
                t = wpool.tile([128, 4096], BF16, tag="w", name=f"w_{name}")
                half = cols // 2
                next_eng().dma_start(t[:, :half], dr[name].ap()[:, :half])
                next_eng().dma_start(t[:, half:cols], dr[name].ap()[:, half:])
                return t

            # Pre-issue every weight-slab DMA in exact use order; the 12-slot
            # pool turns this into a rolling prefetch window.
            _slab_order = ["m1w_0", "m2w_0", "m2w_1"]
            for _i in range(NLAYERS):
                _slab_order += [f"kw{_i}_0", f"kw{_i}_1", f"vw{_i}_0", f"vw{_i}_1",
                                f"qw{_i}_0", f"qw{_i}_1"]
                _slab_order += [f"f1w{_i}_{s}" for s in range(8)]
                _slab_order += [f"f2w{_i}_{s}" for s in range(8)]
            _slab_order += [f"gw_{s}" for s in range(4)]
            _slabs = {}

            def get_slabs(name, S):
                return [_slabs[f"{name}_{s}"] for s in range(S)]

            def linear(dst, name, KC, OC, ops, rhs_fn, N, bias, func):
                """dst[:, oc*N:(oc+1)*N] = func(sum_kc W[kc,oc].T @ rhs(kc) + bias[:,oc])"""
                S = OC // ops
                for s in range(S):
                    slab = _slabs[f"{name}_{s}"]
                    for oi in range(ops):
                        oc = s * ops + oi
                        if N > 64:
                            ps = pp256.tile([128, N], F32, tag="ps256", name=f"ps_{name}_{oc}")
                        else:
                            ps = pp16.tile([128, N], F32, tag="ps16", name=f"ps_{name}_{oc}")
                        for kc in range(KC):
                            off = (oi * KC + kc) * 128
                            nc.tensor.matmul(ps[:, :], slab[:, off:off + 128], rhs_fn(kc),
                                             start=(kc == 0), stop=(kc == KC - 1))
                        nc.scalar.activation(dst[:, oc * N:(oc + 1) * N], ps[:, :], func,
                                             bias=bias[:, oc:oc + 1])

            for _nm in _slab_order:
                _slabs[_nm] = wslab(_nm)

            # ---- timestep embedding: args^T = freqs (x) tsrow -------------
            ps_args = pp256.tile([128, R], F32, tag="ps256")
            nc.tensor.matmul(ps_args[:, :], freqs[:1, :], tsrow[:1, :], start=True, stop=True)
            emb = act.tile([128, 2 * R], BF16, tag="emb")
            nc.scalar.activation(emb[:, 0:R], ps_args[:, :], AF.Sin)
            nc.scalar.activation(emb[:, R:2 * R], ps_args[:, :], AF.Sin, bias=float(np.pi / 2))

            hml = act.tile([128, DC * R], BF16, tag="hml")
            linear(hml, "m1w", 2, 8, 8, lambda kc: emb[:, kc * R:(kc + 1) * R], R, m1b, AF.Silu)
            pe = act.tile([128, DC * R], BF16, tag="pe")
            linear(pe, "m2w", 8, 8, 4, lambda kc: hml[:, kc * R:(kc + 1) * R], R, m2b, AF.Identity)
            kin = act.tile([128, DC * R], BF16, tag="kin")
            nc.vector.tensor_add(kin[:, :], mems[:, :], pe[:, :])

            if debug:
                nc.sync.dma_start(dr["dbg_pe"].ap(), pe[:, :])
                nc.sync.dma_start(dr["dbg_hml"].ap(), hml[:, :])

            def e3(ap_2d, inner=EPC):
                # (p, (a b)) -> (p, a, b) with b innermost (stride 1) of size `inner`
                return ap_2d.rearrange("p (a b) -> p a b", b=inner)

            def layernorm(a, bvec, s_rep, b_rep, out_tag):
                resid = act.tile([128, DC * EPC], F32, tag="resid")
                nc.vector.tensor_add(resid[:, :], a[:, :], bvec[:, :])
                sq = act.tile([128, DC * EPC], F32, tag="sq")
                nc.scalar.activation(sq[:, :], resid[:, :], AF.Square)
                ps_s = pprow.tile([1, 256], F32, tag="psrow")
                nc.tensor.matmul(ps_s[:1, 0:128], onescol[:, :1], resid[:, :], start=True, stop=True)
                nc.tensor.matmul(ps_s[:1, 128:256], onescol[:, :1], sq[:, :], start=True, stop=True)
                sums = act.tile([1, 32], F32, tag="lnsums")
                # col = c*16+e; sum over c (stride 16)
                nc.vector.tensor_reduce(sums[:1, 0:16], ps_s[:1, 0:128].rearrange("p (c e) -> p e c", e=EPC),
                                        axis=mybir.AxisListType.X, op=OP.add)
                nc.vector.tensor_reduce(sums[:1, 16:32], ps_s[:1, 128:256].rearrange("p (c e) -> p e c", e=EPC),
                                        axis=mybir.AxisListType.X, op=OP.add)
                mean = act.tile([1, EPC], F32, tag="mean")
                nc.scalar.mul(mean[:1, :], sums[:1, 0:16], 1.0 / D)
                msq = act.tile([1, EPC], F32, tag="msq")
                nc.scalar.mul(msq[:1, :], sums[:1, 16:32], 1.0 / D)
                m2t = act.tile([1, EPC], F32, tag="m2t")
                nc.scalar.activation(m2t[:1, :], mean[:1, :], AF.Square)
                var = act.tile([1, EPC], F32, tag="var")
                nc.vector.tensor_sub(var[:1, :], msq[:1, :], m2t[:1, :])
                std = act.tile([1, EPC], F32, tag="std")
                nc.scalar.activation(std[:1, :], var[:1, :], AF.Sqrt, bias=1e-6)
                rstd = act.tile([1, EPC], F32, tag="rstd")
                nc.vector.reciprocal(rstd[:1, :], std[:1, :])
                ps_mr = ppmr.tile([128, 32], F32, tag="psmr")
                nc.tensor.matmul(ps_mr[:, 0:16], onesrow[:1, :], mean[:1, :], start=True, stop=True)
                nc.tensor.matmul(ps_mr[:, 16:32], onesrow[:1, :], rstd[:1, :], start=True, stop=True)
                t1 = act.tile([128, DC * EPC], F32, tag="t1")
                nc.vector.tensor_sub(e3(t1[:, :]), e3(resid[:, :]),
                                     ps_mr[:, 0:16].unsqueeze(1).broadcast_to([128, DC, EPC]))
                t2 = act.tile([128, DC * EPC], F32, tag="t2")
                nc.vector.tensor_mul(e3(t2[:, :]), e3(t1[:, :]),
                                     ps_mr[:, 16:32].unsqueeze(1).broadcast_to([128, DC, EPC]))
                t3 = act.tile([128, DC * EPC], F32, tag="t3")
                nc.vector.tensor_mul(t3[:, :], t2[:, :], s_rep[:, :])
                xn = act.tile([128, DC * EPC], F32, tag=out_tag)
                nc.vector.tensor_add(xn[:, :], t3[:, :], b_rep[:, :])
                return xn

            cur = toks
            cur_bf = toks_bf
            for i in range(NLAYERS):
                k = act.tile([128, DC * R], F32, tag="k")
                linear(k, f"kw{i}", 8, 8, 4, lambda kc: kin[:, kc * R:(kc + 1) * R], R,
                       lb[f"kb{i}"], AF.Identity)
                v = act.tile([128, DC * R], F32, tag="v")
                linear(v, f"vw{i}", 8, 8, 4, lambda kc: mems[:, kc * R:(kc + 1) * R], R,
                       lb[f"vb{i}"], AF.Identity)
                q = act.tile([128, DC * EPC], F32, tag="q")
                linear(q, f"qw{i}", 8, 8, 4,
                       lambda kc, c=cur_bf: c[:, kc * EPC:(kc + 1) * EPC], EPC,
                       lb[f"qb{i}"], AF.Identity)

                # logits[0, e*16+j] = sum_d q[d, e] k[d, e*16+j]
                ps_l = pprow.tile([1, R], F32, tag="psrow")
                for kc in range(DC):
                    prod = act2.tile([128, R], F32, tag="prod", name=f"prod{i}_{kc}")
                    nc.vector.tensor_mul(
                        e3(prod[:, :], inner=L),
                        e3(k[:, kc * R:(kc + 1) * R], inner=L),
                        q[:, kc * EPC:(kc + 1) * EPC].unsqueeze(2).broadcast_to([128, EPC, L]))
                    nc.tensor.matmul(ps_l[:1, :], onescol[:, :1], prod[:, :],
                                     start=(kc == 0), stop=(kc == DC - 1))
                # softmax over j groups of 16 with additive mask
                masked = act.tile([1, R], F32, tag="masked")
                nc.vector.scalar_tensor_tensor(masked[:1, :], ps_l[:1, :], float(1.0 / np.sqrt(D)),
                                               amask[:1, :], op0=OP.mult, op1=OP.add)
                expt = act.tile([1, R], F32, tag="expt")
                nc.scalar.activation(expt[:1, :], masked[:1, :], AF.Exp)
                den = act.tile([1, EPC], F32, tag="den")
                nc.vector.tensor_reduce(den[:1, :], e3(expt[:1, :], inner=L),
                                        axis=mybir.AxisListType.X, op=OP.add)
                rden = act.tile([1, EPC], F32, tag="rden")
                nc.vector.reciprocal(rden[:1, :], den[:1, :])
                wn = act.tile([1, R], F32, tag="wn")
                nc.vector.tensor_mul(e3(wn[:1, :], inner=L), e3(expt[:1, :], inner=L),
                                     rden[:1, :].unsqueeze(2).broadcast_to([1, EPC, L]))
                ps_w = pp256.tile([128, R], F32, tag="ps256")
                nc.tensor.matmul(ps_w[:, :], onesrow[:1, :], wn[:1, :], start=True, stop=True)
                attn = act.tile([128, DC * EPC], F32, tag="attn")
                for c in range(DC):
                    prod2 = act2.tile([128, R], F32, tag="prod2", name=f"prod2_{i}_{c}")
                    nc.vector.tensor_mul(prod2[:, :], v[:, c * R:(c + 1) * R], ps_w[:, :])
                    nc.vector.tensor_reduce(attn[:, c * EPC:(c + 1) * EPC],
                                            e3(prod2[:, :], inner=L),
                                            axis=mybir.AxisListType.X, op=OP.add)

                if debug and i == 0:
                    nc.sync.dma_start(dr["dbg_k0"].ap(), k[:, :])
                    nc.sync.dma_start(dr["dbg_v0"].ap(), v[:, :])
                    nc.sync.dma_start(dr["dbg_q0"].ap(), q[:, :])
                    nc.sync.dma_start(dr["dbg_wn0"].ap(), wn[:1, :])
                    nc.sync.dma_start(dr["dbg_logit0"].ap(), masked[:1, :])

                x1 = layernorm(cur, attn, lb[f"ln1s{i}"], lb[f"ln1b{i}"], f"x{i}a")
                x1_bf = act.tile([128, DC * EPC], BF16, tag=f"x{i}abf")
                nc.scalar.copy(x1_bf[:, :], x1[:, :])
                h = act.tile([128, HC * EPC], BF16, tag="h")
                linear(h, f"f1w{i}", 8, 32, 4,
                       lambda kc, x=x1_bf: x[:, kc * EPC:(kc + 1) * EPC], EPC,
                       lb[f"f1b{i}"], AF.Gelu_apprx_tanh)
                ffn = act.tile([128, DC * EPC], F32, tag="ffn")
                linear(ffn, f"f2w{i}", 32, 8, 1,
                       lambda kc: h[:, kc * EPC:(kc + 1) * EPC], EPC,
                       lb[f"f2b{i}"], AF.Identity)
                cur = layernorm(x1, ffn, lb[f"ln2s{i}"], lb[f"ln2b{i}"], f"x{i}b")
                cur_bf = act.tile([128, DC * EPC], BF16, tag=f"x{i}bbf")
                nc.scalar.copy(cur_bf[:, :], cur[:, :])
                if debug and i == 0:
                    nc.sync.dma_start(dr["dbg_x0a"].ap(), x1[:, :])
                    nc.sync.dma_start(dr["dbg_x0b"].ap(), cur[:, :])

            # retrieved = toks + flag * (cur - toks)
            dlt = act.tile([128, DC * EPC], F32, tag="dlt")
            nc.vector.tensor_sub(dlt[:, :], cur[:, :], toks[:, :])
            dltf = act.tile([128, DC * EPC], F32, tag="dltf")
            nc.vector.tensor_mul(e3(dltf[:, :]), e3(dlt[:, :]),
                                 flag[:, :].unsqueeze(1).broadcast_to([128, DC, EPC]))
            retr = act.tile([128, DC * EPC], F32, tag="retr")
            nc.vector.tensor_add(retr[:, :], toks[:, :], dltf[:, :])
            retr_bf = act.tile([128, DC * EPC], BF16, tag="retrbf")
            nc.scalar.copy(retr_bf[:, :], retr[:, :])
            if debug:
                nc.sync.dma_start(dr["dbg_retr"].ap(), retr[:, :])

            # gate: sc = sigmoid(gW^T @ [wm; retr] + gb)
            sc = act.tile([128, DC * EPC], F32, tag="sc")
            linear(sc, "gw", 16, 8, 2,
                   lambda kc: (toks_bf[:, kc * EPC:(kc + 1) * EPC] if kc < 8
                               else retr_bf[:, (kc - 8) * EPC:(kc - 7) * EPC]), EPC,
                   gb, AF.Sigmoid)

            # fused = retr + sc * (toks - retr)
            d2 = act.tile([128, DC * EPC], F32, tag="d2")
            nc.vector.tensor_sub(d2[:, :], toks[:, :], retr[:, :])
            d3 = act.tile([128, DC * EPC], F32, tag="d3")
            nc.vector.tensor_mul(d3[:, :], d2[:, :], sc[:, :])
            fused = act.tile([128, DC * EPC], F32, tag="fused")
            nc.vector.tensor_add(fused[:, :], retr[:, :], d3[:, :])
            nc.sync.dma_start(dr["outp"].ap(), fused[:, :])

    nc.compile()
    return nc


def _get_program(debug=False):
    key = ("prog", debug)
    if key not in _CACHE:
        _CACHE[key] = _build_program(debug)
    return _CACHE[key]


# ------------------------------------------------------------------- driver

def _prepare_inputs(tokens, episode_ids, timesteps, bank_feats, bank_timesteps,
                    bank_counts, params):
    tokens = np.asarray(tokens, np.float32)
    mems, mts, cnts = _resolve_memories(
        tokens, np.asarray(episode_ids), np.asarray(timesteps, np.float32),
        np.asarray(bank_feats, np.float32), np.asarray(bank_timesteps, np.float32),
        np.asarray(bank_counts))

    p = {k: np.asarray(v, np.float32) for k, v in params.items()}
    shared = {}
    shared["freqs"] = np.exp(-np.log(10000.0) * np.arange(HALF, dtype=np.float64) / HALF
                             ).astype(np.float32).reshape(1, HALF)
    shared["onesrow"] = np.ones((1, 128), np.float32)
    shared["onescol"] = np.ones((128, 1), np.float32)
    for nm, W, KC, OC, ops in [("m1w", p["m1W"], 2, 8, 8), ("m2w", p["m2W"], 8, 8, 4),
                               ("gw", p["gW"], 16, 8, 2)]:
        for s, sl in enumerate(_pack_w(W, KC, OC, ops)):
            shared[f"{nm}_{s}"] = sl
    shared["m1b"] = _pack_b(p["m1b"], 8)
    shared["m2b"] = _pack_b(p["m2b"], 8)
    shared["gb"] = _pack_b(p["gb"], 8)
    for i in range(NLAYERS):
        for nm, key, KC, OC, ops in [("qw", f"b{i}_qW", 8, 8, 4), ("kw", f"b{i}_kW", 8, 8, 4),
                                     ("vw", f"b{i}_vW", 8, 8, 4), ("f1w", f"b{i}_f1W", 8, 32, 4),
                                     ("f2w", f"b{i}_f2W", 32, 8, 1)]:
            for s, sl in enumerate(_pack_w(p[key], KC, OC, ops)):
                shared[f"{nm}{i}_{s}"] = sl
        shared[f"qb{i}"] = _pack_b(p[f"b{i}_qb"], 8)
        shared[f"kb{i}"] = _pack_b(p[f"b{i}_kb"], 8)
        shared[f"vb{i}"] = _pack_b(p[f"b{i}_vb"], 8)
        shared[f"f1b{i}"] = _pack_b(p[f"b{i}_f1b"], 32)
        shared[f"f2b{i}"] = _pack_b(p[f"b{i}_f2b"], 8)
        shared[f"ln1s{i}"] = _pack_ln(p[f"b{i}_ln1s"])
        shared[f"ln1b{i}"] = _pack_ln(p[f"b{i}_ln1b"])
        shared[f"ln2s{i}"] = _pack_ln(p[f"b{i}_ln2s"])
        shared[f"ln2b{i}"] = _pack_ln(p[f"b{i}_ln2b"])

    in_maps = []
    for c in range(NCORES):
        sl = slice(c * EPC, (c + 1) * EPC)
        mflat = mems[sl].reshape(R, D)                        # (256, 1024)
        m = dict(shared)
        m["mems"] = _feature_major(mflat).astype(NP_BF16)
        m["toks"] = _feature_major(tokens[sl, 0])
        m["toks_bf"] = m["toks"].astype(NP_BF16)
        m["tsrow"] = np.ascontiguousarray(mts[sl].reshape(1, R))
        cc = cnts[sl]
        j = np.arange(L)
        am = np.where(j[None, :] < np.maximum(cc, 1)[:, None], 0.0, -1e30).astype(np.float32)
        m["amask"] = np.ascontiguousarray(am.reshape(1, R))
        fl = (cc > 0).astype(np.float32)
        m["flag"] = np.ascontiguousarray(np.broadcast_to(fl[None, :], (128, EPC)))
        in_maps.append(m)
    return in_maps


def _assemble_output(results):
    out = np.zeros((B, 1, D), np.float32)
    for c in range(NCORES):
        r = np.asarray(results[c]["outp"])                     # (128, 128)
        oc = r.reshape(128, DC, EPC).transpose(2, 1, 0).reshape(EPC, D)
        out[c * EPC:(c + 1) * EPC, 0] = oc
    return out


def _run(inputs, trace=False, debug=False, tmpdir=None):
    nc = _get_program(debug)
    in_maps = _prepare_inputs(**inputs)
    res = run_bass_kernel_spmd(nc, in_maps, list(range(NCORES)), trace=trace, tmpdir=tmpdir)
    return _assemble_output(res.results), res


def kernel(tokens, episode_ids, timesteps, bank_feats, bank_timesteps, bank_counts, params):
    out, _ = _run(dict(tokens=tokens, episode_ids=episode_ids, timesteps=timesteps,
                       bank_feats=bank_feats, bank_timesteps=bank_timesteps,
                       bank_counts=bank_counts, params=params))
    return out


# revision 33
# speedup vs baseline: 1.0355x; 1.0355x over previous
"""Trainium2 Bass kernel for nn_CogMemBank (episodic memory bank retrieval).

Contract: kernel(**inputs) takes FULL unsharded inputs (as produced by
setup_inputs) and returns the FULL (B, 1, D) output.

Strategy
--------
The reference is a sequential scan over B=128 batch elements, but the bank
update appends the *raw input token* (not a computed value), so the bank
evolution is fully determined by the inputs. The per-element effective memory
(L, D) view is resolved on the host with pure indexing (routing/gather), after
which all 128 retrieval+gate computations are independent.

Device: data-parallel over batch elements, 16 per core on 8 NeuronCores, with
the (replicated) attention/gate/PE-MLP weights streamed from HBM. Activations
live feature-major (features on SBUF partitions, elements on the free dim) so
every linear layer is out^T[oc] = sum_kc W[kc,oc]^T @ x^T[kc] with the weight
chunk as the stationary matmul operand. LayerNorm/softmax cross-partition
reductions use ones-vector matmuls; per-element broadcasts use K=1 matmuls
into PSUM.
"""
import sys
import numpy as np

for _p in ("/opt/trn_rl_repo",):
    if _p not in sys.path:
        sys.path.append(_p)

import ml_dtypes  # noqa: E402
import concourse.bass as bass  # noqa: E402
import concourse.tile as tile  # noqa: E402
from concourse import bacc, mybir  # noqa: E402
from concourse.bass_utils import run_bass_kernel_spmd  # noqa: E402

B, D, L, E = 128, 1024, 16, 4096
FREQ = D // 4          # 256
HALF = FREQ // 2       # 128
NLAYERS = 2
NCORES = 8
EPC = B // NCORES      # 16 elements per core
R = EPC * L            # 256 memory rows per core
DC = D // 128          # 8 feature chunks
HC = (4 * D) // 128    # 32 ffn hidden chunks
F32 = mybir.dt.float32
BF16 = mybir.dt.bfloat16
NP_BF16 = ml_dtypes.bfloat16

_CACHE = {}


# ---------------------------------------------------------------- host side

def _resolve_memories(tokens, episode_ids, timesteps, bank_feats, bank_timesteps, bank_counts):
    """Per-element effective (L, D) memory, (L,) timesteps and count, honoring
    the sequential append/FIFO semantics of the reference scan."""
    feats_state, ts_state, cnt_state = {}, {}, {}
    mems = np.zeros((B, L, D), np.float32)
    mts = np.zeros((B, L), np.float32)
    cnts = np.zeros(B, np.int64)
    for b in range(B):
        eid = int(episode_ids[b])
        if eid not in feats_state:
            feats_state[eid] = np.array(bank_feats[eid], np.float32)
            ts_state[eid] = np.array(bank_timesteps[eid], np.float32)
            cnt_state[eid] = int(bank_counts[eid])
        f, t, c = feats_state[eid], ts_state[eid], cnt_state[eid]
        mems[b] = f
        mts[b] = t
        cnts[b] = c
        tok = tokens[b, 0]
        if c < L:
            f[c] = tok
            t[c] = timesteps[b]
            cnt_state[eid] = c + 1
        else:
            f[:-1] = f[1:]
            f[-1] = tok
            t[:-1] = t[1:]
            t[-1] = timesteps[b]
    return mems, mts, cnts


def _pack_w(Wm, KC, OC, ops):
    """Weight (KC*128, OC*128) -> list of slab arrays (128, ops*KC*128) where
    slab s covers oc in [s*ops, (s+1)*ops) and col = oi*(KC*128) + kc*128 + m,
    i.e. slab[p, oi*KC*128 + kc*128 + m] = W[kc*128+p, (s*ops+oi)*128+m]."""
    Wm = np.asarray(Wm, np.float32)
    arr = Wm.reshape(KC, 128, OC, 128).transpose(2, 1, 0, 3)       # (OC,128,KC,128)
    arr = arr.reshape(OC, 128, KC * 128)
    S = OC // ops
    arr = arr.reshape(S, ops, 128, KC * 128).transpose(0, 2, 1, 3)  # (S,128,ops,KC*128)
    return [np.ascontiguousarray(arr[s].reshape(128, ops * KC * 128).astype(NP_BF16))
            for s in range(S)]


def _pack_b(b, OC):
    return np.ascontiguousarray(np.asarray(b, np.float32).reshape(OC, 128).T)


def _pack_ln(s):
    # (D,) -> (128, 128): [p, c*16+e] = s[c*128+p]  (replicated over e)
    a = np.asarray(s, np.float32).reshape(DC, 128).T          # (128, 8)
    return np.ascontiguousarray(
        np.broadcast_to(a[:, :, None], (128, DC, EPC)).reshape(128, DC * EPC))


def _feature_major(x):
    """(rows, D) -> (128, DC*rows): [p, c*rows + r] = x[r, c*128+p]."""
    rows = x.shape[0]
    return np.ascontiguousarray(
        np.asarray(x, np.float32).reshape(rows, DC, 128).transpose(2, 1, 0).reshape(128, DC * rows))


# ------------------------------------------------------------- bass program

def _build_program(debug=False):
    nc = bacc.Bacc("TRN2", target_bir_lowering=False, debug=False)

    # Register extra const scalars used as activation biases (pi/2 for cos via
    # sin, 1e-6 for the layernorm eps) — same mechanism as the builtin 0.0/1.0.
    def _reg_const(val):
        t = nc.alloc_sbuf_tensor(f"const-f32-{val}", [128, 1], F32)
        nc.gpsimd.memset(t.ap(), val)
        nc.const_aps.aps[(F32, float(val))] = t.ap()

    _reg_const(float(np.pi / 2))
    _reg_const(1e-6)
    nc.all_engine_barrier()

    dr = {}
    dshape = {}

    def din(name, shape, dt=F32):
        dr[name] = nc.dram_tensor(name, list(shape), dt, kind="ExternalInput")
        dshape[name] = tuple(shape)

    def dout(name, shape):
        dr[name] = nc.dram_tensor(name, list(shape), F32, kind="ExternalOutput")
        dshape[name] = tuple(shape)

    # per-core data
    din("mems", (128, DC * R), BF16)
    din("toks", (128, DC * EPC))
    din("toks_bf", (128, DC * EPC), BF16)
    din("tsrow", (1, R))
    din("amask", (1, R))
    din("flag", (128, EPC))
    # shared constants
    din("freqs", (1, HALF))
    din("onesrow", (1, 128))
    din("onescol", (128, 1))
    # weights (slabbed) + biases + ln params
    WSPECS = {"m1w": (2, 8, 8), "m2w": (8, 8, 4), "gw": (16, 8, 2)}
    for i in range(NLAYERS):
        WSPECS[f"qw{i}"] = (8, 8, 4)
        WSPECS[f"kw{i}"] = (8, 8, 4)
        WSPECS[f"vw{i}"] = (8, 8, 4)
        WSPECS[f"f1w{i}"] = (8, 32, 4)
        WSPECS[f"f2w{i}"] = (32, 8, 1)
    for name, (KC, OC, ops) in WSPECS.items():
        for s in range(OC // ops):
            din(f"{name}_{s}", (128, ops * KC * 128), BF16)
    din("m1b", (128, 8))
    din("m2b", (128, 8))
    din("gb", (128, 8))
    for i in range(NLAYERS):
        for nm in ("qb", "kb", "vb", "f2b"):
            din(f"{nm}{i}", (128, 8))
        din(f"f1b{i}", (128, 32))
        for nm in ("ln1s", "ln1b", "ln2s", "ln2b"):
            din(f"{nm}{i}", (128, DC * EPC))

    dout("outp", (128, DC * EPC))
    if debug:
        for nm, shape in [("dbg_k0", (128, DC * R)),
                          ("dbg_v0", (128, DC * R)), ("dbg_wn0", (1, R)),
                          ("dbg_x0a", (128, DC * EPC)), ("dbg_x0b", (128, DC * EPC)),
                          ("dbg_retr", (128, DC * EPC)),
                          ("dbg_q0", (128, DC * EPC)), ("dbg_logit0", (1, R))]:
            dout(nm, shape)
        for nm, shape in [("dbg_pe", (128, DC * R)), ("dbg_hml", (128, DC * R))]:
            dr[nm] = nc.dram_tensor(nm, list(shape), BF16, kind="ExternalOutput")
            dshape[nm] = tuple(shape)

    AF = mybir.ActivationFunctionType
    OP = mybir.AluOpType

    with tile.TileContext(nc) as tc:
        from contextlib import ExitStack
        with ExitStack() as ctx:
            const = ctx.enter_context(tc.tile_pool(name="const", bufs=1))
            act = ctx.enter_context(tc.tile_pool(name="act", bufs=1))
            act2 = ctx.enter_context(tc.tile_pool(name="act2", bufs=2))
            wpool = ctx.enter_context(tc.tile_pool(name="w", bufs=12))
            pp256 = ctx.enter_context(tc.tile_pool(name="pp256", bufs=3, space="PSUM"))
            pprow = ctx.enter_context(tc.tile_pool(name="pprow", bufs=2, space="PSUM"))
            pp16 = ctx.enter_context(tc.tile_pool(name="pp16", bufs=2, space="PSUM"))
            ppmr = ctx.enter_context(tc.tile_pool(name="ppmr", bufs=1, space="PSUM"))

            dma_engines = [nc.sync, nc.scalar, nc.gpsimd]
            dma_rr = [0]

            def next_eng():
                e = dma_engines[dma_rr[0] % len(dma_engines)]
                dma_rr[0] += 1
                return e

            def cload(name, shape, dt=F32):
                t = const.tile(list(shape), dt, tag=name, name=f"c_{name}")
                nc.sync.dma_start(t[:, :], dr[name].ap())
                return t

            mems = cload("mems", (128, DC * R), BF16)
            toks = cload("toks", (128, DC * EPC))
            toks_bf = cload("toks_bf", (128, DC * EPC), BF16)
            tsrow = cload("tsrow", (1, R))
            amask = cload("amask", (1, R))
            flag = cload("flag", (128, EPC))
            freqs = cload("freqs", (1, HALF))
            onesrow = cload("onesrow", (1, 128))
            onescol = cload("onescol", (128, 1))
            m1b = cload("m1b", (128, 8))
            m2b = cload("m2b", (128, 8))
            gb = cload("gb", (128, 8))
            lb = {}
            for i in range(NLAYERS):
                for nm in ("qb", "kb", "vb", "f2b"):
                    lb[f"{nm}{i}"] = cload(f"{nm}{i}", (128, 8))
                lb[f"f1b{i}"] = cload(f"f1b{i}", (128, 32))
                for nm in ("ln1s", "ln1b", "ln2s", "ln2b"):
                    lb[f"{nm}{i}"] = cload(f"{nm}{i}", (128, DC * EPC))

            def wslab(name):
                cols = dshape[name][1]
                t = wpool.tile([128, 4096], BF16, tag="w", name=f"w_{name}")
                half = cols // 2
                next_eng().dma_start(t[:, :half], dr[name].ap()[:, :half])
                next_eng().dma_start(t[:, half:cols], dr[name].ap()[:, half:])
                return t

            def linear(dst, name, KC, OC, ops, rhs_fn, N, bias, func):
                """dst[:, oc*N:(oc+1)*N] = func(sum_kc W[kc,oc].T @ rhs(kc) + bias[:,oc])"""
                S = OC // ops
                for s in range(S):
                    slab = wslab(f"{name}_{s}")
                    for oi in range(ops):
                        oc = s * ops + oi
                        if N > 64:
                            ps = pp256.tile([128, N], F32, tag="ps256", name=f"ps_{name}_{oc}")
                        else:
                            ps = pp16.tile([128, N], F32, tag="ps16", name=f"ps_{name}_{oc}")
                        for kc in range(KC):
                            off = (oi * KC + kc) * 128
                            nc.tensor.matmul(ps[:, :], slab[:, off:off + 128], rhs_fn(kc),
                                             start=(kc == 0), stop=(kc == KC - 1))
                        nc.scalar.activation(dst[:, oc * N:(oc + 1) * N], ps[:, :], func,
                                             bias=bias[:, oc:oc + 1])

            # ---- timestep embedding: args^T = freqs (x) tsrow -------------
            ps_args = pp256.tile([128, R], F32, tag="ps256")
            nc.tensor.matmul(ps_args[:, :], freqs[:1, :], tsrow[:1, :], start=True, stop=True)
            emb = act.tile([128, 2 * R], BF16, tag="emb")
            nc.scalar.activation(emb[:, 0:R], ps_args[:, :], AF.Sin)
            nc.scalar.activation(emb[:, R:2 * R], ps_args[:, :], AF.Sin, bias=float(np.pi / 2))

            hml = act.tile([128, DC * R], BF16, tag="hml")
            linear(hml, "m1w", 2, 8, 8, lambda kc: emb[:, kc * R:(kc + 1) * R], R, m1b, AF.Silu)
            pe = act.tile([128, DC * R], BF16, tag="pe")
            linear(pe, "m2w", 8, 8, 4, lambda kc: hml[:, kc * R:(kc + 1) * R], R, m2b, AF.Identity)
            kin = act.tile([128, DC * R], BF16, tag="kin")
            nc.vector.tensor_add(kin[:, :], mems[:, :], pe[:, :])

            if debug:
                nc.sync.dma_start(dr["dbg_pe"].ap(), pe[:, :])
                nc.sync.dma_start(dr["dbg_hml"].ap(), hml[:, :])

            def e3(ap_2d, inner=EPC):
                # (p, (a b)) -> (p, a, b) with b innermost (stride 1) of size `inner`
                return ap_2d.rearrange("p (a b) -> p a b", b=inner)

            def layernorm(a, bvec, s_rep, b_rep, out_tag):
                resid = act.tile([128, DC * EPC], F32, tag="resid")
                nc.vector.tensor_add(resid[:, :], a[:, :], bvec[:, :])
                sq = act.tile([128, DC * EPC], F32, tag="sq")
                nc.scalar.activation(sq[:, :], resid[:, :], AF.Square)
                ps_s = pprow.tile([1, 256], F32, tag="psrow")
                nc.tensor.matmul(ps_s[:1, 0:128], onescol[:, :1], resid[:, :], start=True, stop=True)
                nc.tensor.matmul(ps_s[:1, 128:256], onescol[:, :1], sq[:, :], start=True, stop=True)
                sums = act.tile([1, 32], F32, tag="lnsums")
                # col = c*16+e; sum over c (stride 16)
                nc.vector.tensor_reduce(sums[:1, 0:16], ps_s[:1, 0:128].rearrange("p (c e) -> p e c", e=EPC),
                                        axis=mybir.AxisListType.X, op=OP.add)
                nc.vector.tensor_reduce(sums[:1, 16:32], ps_s[:1, 128:256].rearrange("p (c e) -> p e c", e=EPC),
                                        axis=mybir.AxisListType.X, op=OP.add)
                mean = act.tile([1, EPC], F32, tag="mean")
                nc.scalar.mul(mean[:1, :], sums[:1, 0:16], 1.0 / D)
                msq = act.tile([1, EPC], F32, tag="msq")
                nc.scalar.mul(msq[:1, :], sums[:1, 16:32], 1.0 / D)
                m2t = act.tile([1, EPC], F32, tag="m2t")
                nc.scalar.activation(m2t[:1, :], mean[:1, :], AF.Square)
                var = act.tile([1, EPC], F32, tag="var")
                nc.vector.tensor_sub(var[:1, :], msq[:1, :], m2t[:1, :])
                std = act.tile([1, EPC], F32, tag="std")
                nc.scalar.activation(std[:1, :], var[:1, :], AF.Sqrt, bias=1e-6)
                rstd = act.tile([1, EPC], F32, tag="rstd")
                nc.vector.reciprocal(rstd[:1, :], std[:1, :])
                ps_mr = ppmr.tile([128, 32], F32, tag="psmr")
                nc.tensor.matmul(ps_mr[:, 0:16], onesrow[:1, :], mean[:1, :], start=True, stop=True)
                nc.tensor.matmul(ps_mr[:, 16:32], onesrow[:1, :], rstd[:1, :], start=True, stop=True)
                t1 = act.tile([128, DC * EPC], F32, tag="t1")
                nc.vector.tensor_sub(e3(t1[:, :]), e3(resid[:, :]),
                                     ps_mr[:, 0:16].unsqueeze(1).broadcast_to([128, DC, EPC]))
                t2 = act.tile([128, DC * EPC], F32, tag="t2")
                nc.vector.tensor_mul(e3(t2[:, :]), e3(t1[:, :]),
                                     ps_mr[:, 16:32].unsqueeze(1).broadcast_to([128, DC, EPC]))
                t3 = act.tile([128, DC * EPC], F32, tag="t3")
                nc.vector.tensor_mul(t3[:, :], t2[:, :], s_rep[:, :])
                xn = act.tile([128, DC * EPC], F32, tag=out_tag)
                nc.vector.tensor_add(xn[:, :], t3[:, :], b_rep[:, :])
                return xn

            cur = toks
            cur_bf = toks_bf
            for i in range(NLAYERS):
                k = act.tile([128, DC * R], F32, tag="k")
                linear(k, f"kw{i}", 8, 8, 4, lambda kc: kin[:, kc * R:(kc + 1) * R], R,
                       lb[f"kb{i}"], AF.Identity)
                v = act.tile([128, DC * R], F32, tag="v")
                linear(v, f"vw{i}", 8, 8, 4, lambda kc: mems[:, kc * R:(kc + 1) * R], R,
                       lb[f"vb{i}"], AF.Identity)
                q = act.tile([128, DC * EPC], F32, tag="q")
                linear(q, f"qw{i}", 8, 8, 4,
                       lambda kc, c=cur_bf: c[:, kc * EPC:(kc + 1) * EPC], EPC,
                       lb[f"qb{i}"], AF.Identity)

                # logits[0, e*16+j] = sum_d q[d, e] k[d, e*16+j]
                ps_l = pprow.tile([1, R], F32, tag="psrow")
                for kc in range(DC):
                    prod = act2.tile([128, R], F32, tag="prod", name=f"prod{i}_{kc}")
                    nc.vector.tensor_mul(
                        e3(prod[:, :], inner=L),
                        e3(k[:, kc * R:(kc + 1) * R], inner=L),
                        q[:, kc * EPC:(kc + 1) * EPC].unsqueeze(2).broadcast_to([128, EPC, L]))
                    nc.tensor.matmul(ps_l[:1, :], onescol[:, :1], prod[:, :],
                                     start=(kc == 0), stop=(kc == DC - 1))
                # softmax over j groups of 16 with additive mask
                masked = act.tile([1, R], F32, tag="masked")
                nc.vector.scalar_tensor_tensor(masked[:1, :], ps_l[:1, :], float(1.0 / np.sqrt(D)),
                                               amask[:1, :], op0=OP.mult, op1=OP.add)
                expt = act.tile([1, R], F32, tag="expt")
                nc.scalar.activation(expt[:1, :], masked[:1, :], AF.Exp)
                den = act.tile([1, EPC], F32, tag="den")
                nc.vector.tensor_reduce(den[:1, :], e3(expt[:1, :], inner=L),
                                        axis=mybir.AxisListType.X, op=OP.add)
                rden = act.tile([1, EPC], F32, tag="rden")
                nc.vector.reciprocal(rden[:1, :], den[:1, :])
                wn = act.tile([1, R], F32, tag="wn")
                nc.vector.tensor_mul(e3(wn[:1, :], inner=L), e3(expt[:1, :], inner=L),
                                     rden[:1, :].unsqueeze(2).broadcast_to([1, EPC, L]))
                ps_w = pp256.tile([128, R], F32, tag="ps256")
                nc.tensor.matmul(ps_w[:, :], onesrow[:1, :], wn[:1, :], start=True, stop=True)
                attn = act.tile([128, DC * EPC], F32, tag="attn")
                for c in range(DC):
                    prod2 = act2.tile([128, R], F32, tag="prod2", name=f"prod2_{i}_{c}")
                    nc.vector.tensor_mul(prod2[:, :], v[:, c * R:(c + 1) * R], ps_w[:, :])
                    nc.vector.tensor_reduce(attn[:, c * EPC:(c + 1) * EPC],
                                            e3(prod2[:, :], inner=L),
                                            axis=mybir.AxisListType.X, op=OP.add)

                if debug and i == 0:
                    nc.sync.dma_start(dr["dbg_k0"].ap(), k[:, :])
                    nc.sync.dma_start(dr["dbg_v0"].ap(), v[:, :])
                    nc.sync.dma_start(dr["dbg_q0"].ap(), q[:, :])
                    nc.sync.dma_start(dr["dbg_wn0"].ap(), wn[:1, :])
                    nc.sync.dma_start(dr["dbg_logit0"].ap(), masked[:1, :])

                x1 = layernorm(cur, attn, lb[f"ln1s{i}"], lb[f"ln1b{i}"], f"x{i}a")
                x1_bf = act.tile([128, DC * EPC], BF16, tag=f"x{i}abf")
                nc.scalar.copy(x1_bf[:, :], x1[:, :])
                h = act.tile([128, HC * EPC], BF16, tag="h")
                linear(h, f"f1w{i}", 8, 32, 4,
                       lambda kc, x=x1_bf: x[:, kc * EPC:(kc + 1) * EPC], EPC,
                       lb[f"f1b{i}"], AF.Gelu_apprx_tanh)
                ffn = act.tile([128, DC * EPC], F32, tag="ffn")
                linear(ffn, f"f2w{i}", 32, 8, 1,
                       lambda kc: h[:, kc * EPC:(kc + 1) * EPC], EPC,
                       lb[f"f2b{i}"], AF.Identity)
                cur = layernorm(x1, ffn, lb[f"ln2s{i}"], lb[f"ln2b{i}"], f"x{i}b")
                cur_bf = act.tile([128, DC * EPC], BF16, tag=f"x{i}bbf")
                nc.scalar.copy(cur_bf[:, :], cur[:, :])
                if debug and i == 0:
                    nc.sync.dma_start(dr["dbg_x0a"].ap(), x1[:, :])
                    nc.sync.dma_start(dr["dbg_x0b"].ap(), cur[:, :])

            # retrieved = toks + flag * (cur - toks)
            dlt = act.tile([128, DC * EPC], F32, tag="dlt")
            nc.vector.tensor_sub(dlt[:, :], cur[:, :], toks[:, :])
            dltf = act.tile([128, DC * EPC], F32, tag="dltf")
            nc.vector.tensor_mul(e3(dltf[:, :]), e3(dlt[:, :]),
                                 flag[:, :].unsqueeze(1).broadcast_to([128, DC, EPC]))
            retr = act.tile([128, DC * EPC], F32, tag="retr")
            nc.vector.tensor_add(retr[:, :], toks[:, :], dltf[:, :])
            retr_bf = act.tile([128, DC * EPC], BF16, tag="retrbf")
            nc.scalar.copy(retr_bf[:, :], retr[:, :])
            if debug:
                nc.sync.dma_start(dr["dbg_retr"].ap(), retr[:, :])

            # gate: sc = sigmoid(gW^T @ [wm; retr] + gb)
            sc = act.tile([128, DC * EPC], F32, tag="sc")
            linear(sc, "gw", 16, 8, 2,
                   lambda kc: (toks_bf[:, kc * EPC:(kc + 1) * EPC] if kc < 8
                               else retr_bf[:, (kc - 8) * EPC:(kc - 7) * EPC]), EPC,
                   gb, AF.Sigmoid)

            # fused = retr + sc * (toks - retr)
            d2 = act.tile([128, DC * EPC], F32, tag="d2")
            nc.vector.tensor_sub(d2[:, :], toks[:, :], retr[:, :])
            d3 = act.tile([128, DC * EPC], F32, tag="d3")
            nc.vector.tensor_mul(d3[:, :], d2[:, :], sc[:, :])
            fused = act.tile([128, DC * EPC], F32, tag="fused")
            nc.vector.tensor_add(fused[:, :], retr[:, :], d3[:, :])
            nc.sync.dma_start(dr["outp"].ap(), fused[:, :])

    nc.compile()
    return nc


def _get_program(debug=False):
    key = ("prog", debug)
    if key not in _CACHE:
        _CACHE[key] = _build_program(debug)
    return _CACHE[key]


# ------------------------------------------------------------------- driver

def _prepare_inputs(tokens, episode_ids, timesteps, bank_feats, bank_timesteps,
                    bank_counts, params):
    tokens = np.asarray(tokens, np.float32)
    mems, mts, cnts = _resolve_memories(
        tokens, np.asarray(episode_ids), np.asarray(timesteps, np.float32),
        np.asarray(bank_feats, np.float32), np.asarray(bank_timesteps, np.float32),
        np.asarray(bank_counts))

    p = {k: np.asarray(v, np.float32) for k, v in params.items()}
    shared = {}
    shared["freqs"] = np.exp(-np.log(10000.0) * np.arange(HALF, dtype=np.float64) / HALF
                             ).astype(np.float32).reshape(1, HALF)
    shared["onesrow"] = np.ones((1, 128), np.float32)
    shared["onescol"] = np.ones((128, 1), np.float32)
    for nm, W, KC, OC, ops in [("m1w", p["m1W"], 2, 8, 8), ("m2w", p["m2W"], 8, 8, 4),
                               ("gw", p["gW"], 16, 8, 2)]:
        for s, sl in enumerate(_pack_w(W, KC, OC, ops)):
            shared[f"{nm}_{s}"] = sl
    shared["m1b"] = _pack_b(p["m1b"], 8)
    shared["m2b"] = _pack_b(p["m2b"], 8)
    shared["gb"] = _pack_b(p["gb"], 8)
    for i in range(NLAYERS):
        for nm, key, KC, OC, ops in [("qw", f"b{i}_qW", 8, 8, 4), ("kw", f"b{i}_kW", 8, 8, 4),
                                     ("vw", f"b{i}_vW", 8, 8, 4), ("f1w", f"b{i}_f1W", 8, 32, 4),
                                     ("f2w", f"b{i}_f2W", 32, 8, 1)]:
            for s, sl in enumerate(_pack_w(p[key], KC, OC, ops)):
                shared[f"{nm}{i}_{s}"] = sl
        shared[f"qb{i}"] = _pack_b(p[f"b{i}_qb"], 8)
        shared[f"kb{i}"] = _pack_b(p[f"b{i}_kb"], 8)
        shared[f"vb{i}"] = _pack_b(p[f"b{i}_vb"], 8)
        shared[f"f1b{i}"] = _pack_b(p[f"b{i}_f1b"], 32)
        shared[f"f2b{i}"] = _pack_b(p[f"b{i}_f2b"], 8)
        shared[f"ln1s{i}"] = _pack_ln(p[f"b{i}_ln1s"])
        shared[f"ln1b{i}"] = _pack_ln(p[f"b{i}_ln1b"])
        shared[f"ln2s{i}"] = _pack_ln(p[f"b{i}_ln2s"])
        shared[f"ln2b{i}"] = _pack_ln(p[f"b{i}_ln2b"])

    in_maps = []
    for c in range(NCORES):
        sl = slice(c * EPC, (c + 1) * EPC)
        mflat = mems[sl].reshape(R, D)                        # (256, 1024)
        m = dict(shared)
        m["mems"] = _feature_major(mflat).astype(NP_BF16)
        m["toks"] = _feature_major(tokens[sl, 0])
        m["toks_bf"] = m["toks"].astype(NP_BF16)
        m["tsrow"] = np.ascontiguousarray(mts[sl].reshape(1, R))
        cc = cnts[sl]
        j = np.arange(L)
        am = np.where(j[None, :] < np.maximum(cc, 1)[:, None], 0.0, -1e30).astype(np.float32)
        m["amask"] = np.ascontiguousarray(am.reshape(1, R))
        fl = (cc > 0).astype(np.float32)
        m["flag"] = np.ascontiguousarray(np.broadcast_to(fl[None, :], (128, EPC)))
        in_maps.append(m)
    return in_maps


def _assemble_output(results):
    out = np.zeros((B, 1, D), np.float32)
    for c in range(NCORES):
        r = np.asarray(results[c]["outp"])                     # (128, 128)
        oc = r.reshape(128, DC, EPC).transpose(2, 1, 0).reshape(EPC, D)
        out[c * EPC:(c + 1) * EPC, 0] = oc
    return out


def _run(inputs, trace=False, debug=False, tmpdir=None):
    nc = _get_program(debug)
    in_maps = _prepare_inputs(**inputs)
    res = run_bass_kernel_spmd(nc, in_maps, list(range(NCORES)), trace=trace, tmpdir=tmpdir)
    return _assemble_output(res.results), res


def kernel(tokens, episode_ids, timesteps, bank_feats, bank_timesteps, bank_counts, params):
    out, _ = _run(dict(tokens=tokens, episode_ids=episode_ids, timesteps=timesteps,
                       bank_feats=bank_feats, bank_timesteps=bank_timesteps,
                       bank_counts=bank_counts, params=params))
    return out
